# revision 25
# baseline (speedup 1.0000x reference)
"""Self-attention (CrossAttention with context=x) Trainium2 Bass kernel, v2.

Sharding: B*h = 16 head-instances across 8 cores -> each core owns one batch
and 2 heads (A rows 0-63, B rows 64-127 of the stacked qT/kT layout).

Per-core pipeline:
- S^T = K Q^T per head, PE row-tiled: head A uses PE rows 0-63, head B rows
  64-127 -> concurrent matmul pairs (~2x effective column rate, K=dh=64).
- exp split across engines: ScalarE exact exp (fp8e4 out, scale=1/8 folded)
  for head A + every Nth head-B tile; VectorE computes the rest of head B via
  a Schraudolph-style bit trick straight in e4m3 bit space
  (tensor_scalar mult+add -> int8 view of the fp8 tile).
- PV matmul in fp8e4 with perf_mode=DoubleRow (two 128-key tiles per pass),
  V augmented with a ones column -> softmax denominators for free.
- Epilogue: per-query 1/denominator via VectorE reciprocal + GpSimd
  partition_broadcast, prescale O^T, stack both heads into one [128, x]
  lhsT (head B moved to partitions 64-127 by SBUF->SBUF DMA), single
  output-projection matmul per 128-query tile, Y copies split Scalar/Vector.
Host: transpose x per batch, slice per-head weight columns, sum the 4
per-batch partial outputs, add bias.
"""
import sys
sys.path.insert(0, "/opt/trn_rl_repo")

import numpy as np
from contextlib import ExitStack

import concourse.bass as bass
import concourse.tile as tile
from concourse import bacc, mybir
from concourse import bass_utils

DH = 64
D = 512
SEQ = 4096
B = 2
N_CORES = 8

f32 = mybir.dt.float32
bf16 = mybir.dt.bfloat16
fp8 = mybir.dt.float8e3   # e3m4: 4-bit mantissa, P in [0.27, 4] fits range
i8 = mybir.dt.int8
Exp = mybir.ActivationFunctionType.Exp
DR = mybir.MatmulPerfMode.DoubleRow
Mult = mybir.AluOpType.mult
Add = mybir.AluOpType.add

LOG2E = 1.4426950408889634
# DVE exp: e3m4 bits = round(16*(log2e*(s/8)) + 16*3 - c); c ~= 0.688 balances
# the 2^f vs 1+f gap of the linear-mantissa approximation.
A8 = 2.0 * LOG2E
B8 = 48.0 - 0.688
EXPB_SCALAR_MOD = 6  # every 6th head-B exp tile runs on ScalarE instead of DVE

USE_DVE_EXP = True     # Schraudolph bit-trick exp on DVE for most of head B
USE_DOUBLEROW = False  # DoubleRow needs fp8e4/e5; P is e3m4 for precision


def build_nc(N=SEQ):
    nc = bacc.Bacc("TRN2", target_bir_lowering=False, debug=False,
                   num_devices=N_CORES)
    xT = nc.dram_tensor("xT", [D, N], bf16, kind="ExternalInput").ap()
    Wq2 = nc.dram_tensor("Wq2", [D, 128], bf16, kind="ExternalInput").ap()
    Wk2 = nc.dram_tensor("Wk2", [D, 128], bf16, kind="ExternalInput").ap()
    Wv2 = nc.dram_tensor("Wv2", [D, 128], bf16, kind="ExternalInput").ap()
    Wo2 = nc.dram_tensor("Wo2", [128, D], bf16, kind="ExternalInput").ap()
    Y = nc.dram_tensor("Y", [N, D], f32, kind="ExternalOutput").ap()

    KC = N // 128          # 128-key chunks
    PAIRS = KC // 2        # 256-key DoubleRow pairs
    NCH = N // 512         # 512-query chunks
    NQB = N // 1024        # 1024-query blocks
    ND = D // 128

    with tile.TileContext(nc) as tc, ExitStack() as ctx:
        wp = ctx.enter_context(tc.tile_pool(name="weights", bufs=1))
        pp = ctx.enter_context(tc.tile_pool(name="persist", bufs=1))
        xp = ctx.enter_context(tc.tile_pool(name="xload", bufs=1))
        ptp = ctx.enter_context(tc.tile_pool(name="pt", bufs=24))
        rbp = ctx.enter_context(tc.tile_pool(name="rb", bufs=2))
        yop = ctx.enter_context(tc.tile_pool(name="ysb", bufs=4))
        stp = ctx.enter_context(tc.tile_pool(name="stps", bufs=3, space="PSUM"))
        bkp = ctx.enter_context(tc.tile_pool(name="bkps", bufs=2, space="PSUM"))

        qTc = [pp.tile([128, 512], bf16, tag=f"qT{i}", name=f"qT{i}")
               for i in range(NCH)]
        kTc = [pp.tile([128, 512], bf16, tag=f"kT{i}", name=f"kT{i}")
               for i in range(NCH)]
        # V in bf16, padded pair layout: [keys, pair, 2, 80]
        # cols 0-63 = V dims, col 64 = ones (denominator), 65-79 pad.
        VA = pp.tile([128, PAIRS, 2, 80], bf16, tag="VA")
        VB = pp.tile([128, PAIRS, 2, 80], bf16, tag="VB")
        OTsA = pp.tile([65, N], bf16, tag="OTsA")   # rows 0-63 O^T, row 64 sums
        OTsB = pp.tile([65, N], bf16, tag="OTsB")
        OT2 = pp.tile([128, N], bf16, tag="OT2")    # stacked scaled O^T
        wq = wp.tile([128, ND, 128], bf16, tag="wq")
        wk = wp.tile([128, ND, 128], bf16, tag="wk")
        wv = wp.tile([128, ND, 128], bf16, tag="wv")
        wo = wp.tile([128, D], bf16, tag="wo")
        nc.sync.dma_start(wq[:], Wq2.rearrange("(t p) m -> p t m", p=128))
        nc.sync.dma_start(wk[:], Wk2.rearrange("(t p) m -> p t m", p=128))
        nc.sync.dma_start(wv[:], Wv2.rearrange("(t p) m -> p t m", p=128))
        nc.sync.dma_start(wo[:], Wo2)
        nc.vector.memset(VA[:, :, :, 64:65], 1.0)
        nc.vector.memset(VB[:, :, :, 64:65], 1.0)

        xTd = xT.rearrange("(t p) n -> p t n", p=128)
        xts = []
        for t in range(ND):
            xt_t = xp.tile([128, N], bf16, tag=f"x{t}")
            nc.sync.dma_start(xt_t[:], xTd[:, t, :])
            xts.append(xt_t)

        def proj_qk(c, wmat, dst, on_scalar):
            ps = bkp.tile([128, 512], f32, tag="bk", name=f"pj{wmat.name}_{c}")
            for d in range(ND):
                nc.tensor.matmul(ps[:], wmat[:, d, :],
                                 xts[d][:, c * 512:(c + 1) * 512],
                                 start=(d == 0), stop=(d == ND - 1))
            if on_scalar:
                nc.scalar.copy(dst[:], ps[:])
            else:
                nc.vector.tensor_copy(dst[:], ps[:])

        def proj_v(kc):
            psv = bkp.tile([128, 128], f32, tag="bk", name=f"pv{kc}")
            for d in range(ND):
                nc.tensor.matmul(psv[:], xts[d][:, kc * 128:(kc + 1) * 128],
                                 wv[:, d, :], start=(d == 0), stop=(d == ND - 1))
            nc.vector.tensor_copy(VA[:, kc // 2, kc % 2, 0:64], psv[:, 0:64])
            nc.vector.tensor_copy(VB[:, kc // 2, kc % 2, 0:64], psv[:, 64:128])

        # prefix: kT fully (qb0 needs all keys), qT chunks 0-1
        for c in range(NCH):
            proj_qk(c, wk, kTc[c], on_scalar=True)
        for c in range(2):
            proj_qk(c, wq, qTc[c], on_scalar=True)

        def pv_group(qb, h, j, ptd, V, dst):
            def run():
                po = bkp.tile([65, 512], f32, tag="bk", name=f"po{qb}_{h}_{j}")
                if USE_DOUBLEROW:
                    for p in range(PAIRS):
                        nc.tensor.matmul(po[:], V[:, p, :, 0:65],
                                         ptd[p][:, :, j * 512:(j + 1) * 512],
                                         start=(p == 0), stop=(p == PAIRS - 1),
                                         perf_mode=DR)
                else:
                    for p in range(PAIRS):
                        for i in (0, 1):
                            nc.tensor.matmul(
                                po[:], V[:, p, i, 0:65],
                                ptd[p][:, i, j * 512:(j + 1) * 512],
                                start=(p == 0 and i == 0),
                                stop=(p == PAIRS - 1 and i == 1))
                q0 = qb * 1024 + j * 512
                if j == 0:
                    nc.scalar.copy(dst[:, q0:q0 + 512], po[:])
                else:
                    nc.vector.tensor_copy(dst[:, q0:q0 + 512], po[:])
            return run

        def epilogue(qb):
            qcols = slice(qb * 1024, (qb + 1) * 1024)
            # stage the sums rows at partition 0 (partition_broadcast reads
            # the source tile's partition 0) via SBUF->SBUF DMA
            sa = rbp.tile([1, 1024], bf16, tag="srA", name=f"srA{qb}")
            sb = rbp.tile([1, 1024], bf16, tag="srB", name=f"srB{qb}")
            nc.sync.dma_start(sa[:], OTsA[64:65, qcols])
            nc.sync.dma_start(sb[:], OTsB[64:65, qcols])
            with nc.allow_low_precision(reason="1/denominator in bf16 is ~0.4% rel, within tolerance"):
                nc.vector.reciprocal(sa[:], sa[:])
                nc.vector.reciprocal(sb[:], sb[:])
            ra = rbp.tile([64, 1024], bf16, tag="rbA", name=f"rbA{qb}")
            rb = rbp.tile([64, 1024], bf16, tag="rbB", name=f"rbB{qb}")
            nc.gpsimd.partition_broadcast(ra[:], sa[:])
            nc.gpsimd.partition_broadcast(rb[:], sb[:])
            nc.vector.tensor_mul(OTsA[0:64, qcols], OTsA[0:64, qcols], ra[:])
            nc.vector.tensor_mul(OTsB[0:64, qcols], OTsB[0:64, qcols], rb[:])
            nc.sync.dma_start(OT2[0:64, qcols], OTsA[0:64, qcols])
            nc.sync.dma_start(OT2[64:128, qcols], OTsB[0:64, qcols])
            for m in range(8):
                c0 = qb * 1024 + m * 128
                py = bkp.tile([128, 512], f32, tag="bk", name=f"py{qb}_{m}")
                nc.tensor.matmul(py[:], OT2[:, c0:c0 + 128], wo[:],
                                 start=True, stop=True)
                yo = yop.tile([128, 512], f32, tag="yo", name=f"yo{qb}_{m}")
                nc.scalar.copy(yo[:], py[:])
                nc.sync.dma_start(Y[c0:c0 + 128, :], yo[:])

        prev_work = []   # closures from previous qb: 4 PV groups + epilogue
        for qb in range(NQB):
            ptA = {}
            ptB = {}
            inject = {0: None, 2: None, 4: None, 6: None, 8: None}
            if prev_work:
                inject[0], inject[2], inject[4], inject[6], inject[8] = prev_work
            for kc in range(KC):
                stA = stp.tile([128, 1024], f32, tag="st", name=f"stA{qb}_{kc}")
                stB = stp.tile([128, 1024], f32, tag="st", name=f"stB{qb}_{kc}")
                for j in (0, 1):
                    qc = qb * 2 + j
                    nc.tensor.matmul(
                        stA[:, j * 512:(j + 1) * 512],
                        kTc[kc // 4][0:64, (kc % 4) * 128:(kc % 4 + 1) * 128],
                        qTc[qc][0:64, :], start=True, stop=True)
                    nc.tensor.matmul(
                        stB[:, j * 512:(j + 1) * 512],
                        kTc[kc // 4][64:128, (kc % 4) * 128:(kc % 4 + 1) * 128],
                        qTc[qc][64:128, :], start=True, stop=True)
                pair = kc // 2
                if kc % 2 == 0:
                    ptA[pair] = ptp.tile([128, 2, 1024], fp8, tag="ptA",
                                         name=f"ptA{qb}_{pair}")
                    ptB[pair] = ptp.tile([128, 2, 1024], fp8, tag="ptB",
                                         name=f"ptB{qb}_{pair}")
                nc.scalar.activation(ptA[pair][:, kc % 2, :], stA[:], Exp,
                                     scale=0.125)
                if (not USE_DVE_EXP) or kc % EXPB_SCALAR_MOD == EXPB_SCALAR_MOD - 1:
                    nc.scalar.activation(ptB[pair][:, kc % 2, :], stB[:], Exp,
                                         scale=0.125)
                else:
                    nc.vector.tensor_scalar(
                        ptB[pair][:, kc % 2, :].bitcast(i8), stB[:],
                        A8, B8, Mult, Add)
                if kc in inject and inject[kc] is not None:
                    inject[kc]()
                if qb == 0:
                    proj_v(kc)
                    if kc % 4 == 3 and 2 + kc // 4 < NCH:
                        proj_qk(2 + kc // 4, wq, qTc[2 + kc // 4],
                                on_scalar=(kc // 4 % 2 == 1))
            prev_work = [
                pv_group(qb, 0, 0, ptA, VA, OTsA),
                pv_group(qb, 0, 1, ptA, VA, OTsA),
                pv_group(qb, 1, 0, ptB, VB, OTsB),
                pv_group(qb, 1, 1, ptB, VB, OTsB),
                lambda q=qb: epilogue(q),
            ]
        # drain the last block
        for w in prev_work:
            w()
    nc.compile()
    return nc


_NC_CACHE = {}


def _get_nc(N=SEQ):
    if N not in _NC_CACHE:
        _NC_CACHE[N] = build_nc(N)
    return _NC_CACHE[N]


def kernel(x, Wq, Wk, Wv, Wo, bo):
    x = np.asarray(x, dtype=np.float32)
    Wq = np.asarray(Wq, dtype=np.float32)
    Wk = np.asarray(Wk, dtype=np.float32)
    Wv = np.asarray(Wv, dtype=np.float32)
    Wo = np.asarray(Wo, dtype=np.float32)
    bo = np.asarray(bo, dtype=np.float32)
    Bx, N, Dx = x.shape
    nc = _get_nc(N)
    in_maps = []
    import ml_dtypes
    bfl = ml_dtypes.bfloat16
    xTs = [np.ascontiguousarray(x[b].T).astype(bfl) for b in range(Bx)]
    for c in range(N_CORES):
        b = c // 4
        hA = 2 * (c % 4)
        cols = slice(hA * DH, (hA + 2) * DH)
        in_maps.append({
            "xT": xTs[b],
            "Wq2": np.ascontiguousarray(Wq[:, cols]).astype(bfl),
            "Wk2": np.ascontiguousarray(Wk[:, cols]).astype(bfl),
            "Wv2": np.ascontiguousarray(Wv[:, cols]).astype(bfl),
            "Wo2": np.ascontiguousarray(Wo[cols, :]).astype(bfl),
        })
    res = bass_utils.run_bass_kernel_spmd(nc, in_maps, core_ids=list(range(N_CORES)))
    out = np.zeros((Bx, N, Dx), dtype=np.float32)
    for c in range(N_CORES):
        out[c // 4] += res.results[c]["Y"]
    out += bo
    return out
